# revision 33
# baseline (speedup 1.0000x reference)
"""GraphWaveNet kernel for Trainium2 (Bass/Tile), 8 NeuronCores.

Sharding: 8 cores = 2 slice-groups (batches {0,1} / {2,3}, 64 feats each
packed into one 256B bf16 table row) x 4 node-quarters (2560 nodes each).

Algorithm (only t=11 survives the final 1x1 conv; GCN does not mix time):
- conv stack evaluated at t in {10,11} only, replicated per slice-group,
  producing table0 [10240, 128] bf16 (node-major rows, own-quarter-first
  row order per core).
- GCN layer as (A_norm h) W: edge normalization dsq[src]*dsq[dst] baked
  into one-hot scatter weights P (host-built); self-loop dsq^2 as diagonal
  matmuls. Gather via dma_gather (1024 edges / instruction); scatter via
  feature-major matmuls into PSUM column regions (out = [128 feats, node
  cols]); then W-matmul + relu + transpose back to node-major rows.
- L1 output shards AllGather'd across the 4 quarter-cores of each
  slice-group into table1 (natural row order); L2 consumes it and emits
  y = wo . h2 + bo for the core's own quarter x 2 batches.
"""

import sys

sys.path.insert(0, "/opt/trn_rl_repo")

import numpy as np
import ml_dtypes

B, T, N, FIN, H, E = 4, 12, 10000, 2, 64, 80000
NP = 10240
NBLK = 80        # 128-node blocks
NQ = 4           # node quarters
NBO = 20         # blocks per quarter
QN = NBO * 128   # nodes per quarter (2560)
GSZ = 8          # chunks per dma_gather (8 * 128 = 1024 idx, HW ring limit)
DSL = 128        # table row width: 2 slices x 64 feats

_cache = {}


def _host_prep(inputs):
    x = np.asarray(inputs["x"], dtype=np.float32)
    src = np.asarray(inputs["edge_index"][0], dtype=np.int64)
    dst = np.asarray(inputs["edge_index"][1], dtype=np.int64)
    w1 = np.asarray(inputs["w1"], dtype=np.float32)
    b1 = np.asarray(inputs["b1"], dtype=np.float32)
    w2 = np.asarray(inputs["w2"], dtype=np.float32)
    b2 = np.asarray(inputs["b2"], dtype=np.float32)
    gw1 = np.asarray(inputs["gw1"], dtype=np.float32)
    gb1 = np.asarray(inputs["gb1"], dtype=np.float32)
    gw2 = np.asarray(inputs["gw2"], dtype=np.float32)
    gb2 = np.asarray(inputs["gb2"], dtype=np.float32)
    wo = np.asarray(inputs["wo"], dtype=np.float32)
    bo = float(np.asarray(inputs["bo"], dtype=np.float32)[0])

    deg = np.bincount(dst, minlength=N).astype(np.float64) + 1.0
    dsq = np.ones(NP, dtype=np.float32)
    dsq[:N] = (deg ** -0.5).astype(np.float32)

    order = np.argsort(dst, kind="stable")
    src_s = src[order]
    dst_s = dst[order]

    # 128-node windows (whole blocks): starts into dst-sorted edges
    wbound = np.searchsorted(dst_s, np.arange(0, NP + 128, 128))
    wcnt = wbound[1:] - wbound[:-1]

    # pad per-block edge counts to the max across quarters, then chunk the
    # concatenated stream across block boundaries (uniform structure)
    cnt_max = np.array([max(int(wcnt[q * NBO + i]) for q in range(NQ))
                        for i in range(NBO)], dtype=np.int64)
    start_e = np.zeros(NBO + 1, dtype=np.int64)
    start_e[1:] = np.cumsum(cnt_max)
    tot_e = int(start_e[NBO])
    nchp = -(-tot_e // 128)
    ngr = -(-nchp // GSZ)

    # per block: list of (chunk idx, row range within chunk)
    sched = []
    for i in range(NBO):
        lo, hi = int(start_e[i]), int(start_e[i + 1])
        ovl = []
        for ci in range(lo // 128, (hi - 1) // 128 + 1 if hi > lo else lo // 128):
            k0 = max(0, lo - ci * 128)
            k1 = min(128, hi - ci * 128)
            ovl.append((ci, k0, k1))
        sched.append(ovl)
    npsl = sum(len(o) for o in sched)
    psl_of = []
    acc = 0
    for o in sched:
        psl_of.append(acc)
        acc += len(o)
    chunks = (nchp, sched, psl_of, npsl, start_e)

    # per-core host data
    cores = []
    for c in range(8):
        sg, q = c % 2, c // 2
        own = list(range(NBO * q, NBO * q + NBO))
        blk_perm = own + [b for b in range(NBLK) if not (NBO * q <= b < NBO * q + NBO)]
        rowmap = np.empty(NP, dtype=np.int64)
        for pos, b in enumerate(blk_perm):
            rowmap[b * 128:(b + 1) * 128] = np.arange(pos * 128, (pos + 1) * 128)

        # XOR-region row map for table1: node n -> region (c ^ (2*(n//QN)+sg))>>1
        row2 = np.empty(NP, dtype=np.int64)
        for qq in range(NQ):
            dd = c ^ (2 * qq + sg)
            row2[qq * QN:(qq + 1) * QN] = (dd >> 1) * QN + np.arange(QN)

        nchp, sched, psl_of, npsl, start_e = chunks
        eidx1 = np.zeros((16, ngr * 64), dtype=np.int16)
        eidx2 = np.zeros((16, ngr * 64), dtype=np.int16)
        P = np.zeros((128, npsl * 128), dtype=np.float32)
        for i in range(NBO):
            gwin = q * NBO + i
            e0, e1 = int(wbound[gwin]), int(wbound[gwin + 1])
            for j, (ci, k0, k1) in enumerate(sched[i]):
                pidx = psl_of[i] + j
                base = ci * 128 - int(start_e[i])   # stream pos -> in-block pos
                for k in range(k0, k1):
                    r = base + k
                    if r >= e1 - e0:
                        continue                    # pad slot: idx 0, P 0
                    e = e0 + r
                    pos = ci * 128 + k
                    gi = pos // (GSZ * 128)
                    ii = pos - gi * GSZ * 128
                    eidx1[ii % 16, gi * 64 + ii // 16] = rowmap[src_s[e]]
                    eidx2[ii % 16, gi * 64 + ii // 16] = row2[src_s[e]]
                    P[k, pidx * 128 + int(dst_s[e]) - (q * 2560 + i * 128)] = \
                        dsq[src_s[e]] * dsq[dst_s[e]]
        eidx1 = np.tile(eidx1, (8, 1))
        eidx2 = np.tile(eidx2, (8, 1))

        dsqd = np.zeros((128, NBO * 128), dtype=np.float32)
        for i in range(NBO):
            nn = q * 2560 + i * 128 + np.arange(128)
            dsqd[np.arange(128), i * 128 + np.arange(128)] = dsq[nn] ** 2

        # conv input, blocks in own-first order: rows 8s+r, r in 0..5 =
        # x[b, t, :, cc] for (t, cc) pairs, row 6 = 1.0 (bias), row 7 = 0
        xt = np.zeros((8, 2 * NP), dtype=np.float32)
        for s in range(2):
            bb = 2 * sg + s
            for ti, t in enumerate((9, 10, 11)):
                for cc in range(FIN):
                    col = np.zeros(NP, dtype=np.float32)
                    col[:N] = x[bb, t, :, cc]
                    xt[2 * ti + cc, s * NP:(s + 1) * NP] = col[_perm_nodes(blk_perm)]
            xt[6, s * NP:(s + 1) * NP] = 1.0

        cores.append({
            "eidx1": eidx1, "eidx2": eidx2,
            "P": P.astype(ml_dtypes.bfloat16),
            "dsqd": dsqd.astype(ml_dtypes.bfloat16),
            "xt": xt.astype(ml_dtypes.bfloat16),
        })

    # shared weights
    W1m = np.zeros((6, 64), dtype=np.float32)
    for k in range(3):
        for cc in range(FIN):
            W1m[2 * k + cc, :] = w1[:, cc, 0, k]
    W1ab = np.zeros((8, 128), dtype=np.float32)
    W1ab[0:6, 0:64] = W1m
    W1ab[6, 0:64] = b1
    W1ab[2:6, 64:128] = W1m[0:4]
    W1ab[6, 64:128] = b1
    W2m = np.zeros((128, 64), dtype=np.float32)
    W2m[:64, :] = w2[:, :, 0, 0].T
    W2m[64:, :] = w2[:, :, 0, 1].T

    gwb1 = np.zeros((128, 128), dtype=np.float32)
    gwb1[0:64, 0:64] = gw1
    gwb1[64:128, 64:128] = gw1
    gwb2 = np.zeros((128, 128), dtype=np.float32)
    gwb2[0:64, 0:64] = gw2
    gwb2[64:128, 64:128] = gw2

    shared = {
        "W1ab": W1ab.astype(ml_dtypes.bfloat16),
        "W2m": W2m.astype(ml_dtypes.bfloat16),
        "b2c": b2.reshape(64, 1).astype(np.float32),
        "gwb1": gwb1.astype(ml_dtypes.bfloat16),
        "gwb2": gwb2.astype(ml_dtypes.bfloat16),
        "gbd1": np.concatenate([gb1, gb1]).reshape(128, 1).astype(np.float32),
        "gbd2": np.concatenate([gb2, gb2]).reshape(128, 1).astype(np.float32),
        "wod": np.tile(wo[0, :, 0, 0], 2).reshape(128, 1).astype(ml_dtypes.bfloat16),
    }
    in_maps = [dict(shared, **cores[c]) for c in range(8)]
    return in_maps, chunks, ngr, bo


def _perm_nodes(blk_perm):
    p = np.empty(NP, dtype=np.int64)
    for pos, b in enumerate(blk_perm):
        p[pos * 128:(pos + 1) * 128] = np.arange(b * 128, (b + 1) * 128)
    return p


def _build(chunks, ngr, bo_f):
    from concourse import bass, bacc, tile
    from concourse.masks import make_identity
    import mybir

    f32, bf16, i16 = mybir.dt.float32, mybir.dt.bfloat16, mybir.dt.int16
    nchp, sched, psl_of, npsl, start_e = chunks

    nc = bacc.Bacc("TRN2", target_bir_lowering=False, debug=False,
                   num_devices=8)

    ext = {}
    for name, shape, dt in [
        ("xt", [8, 2 * NP], bf16), ("W1ab", [8, 128], bf16), ("W2m", [128, 64], bf16),
        ("b2c", [64, 1], f32), ("gwb1", [128, 128], bf16), ("gwb2", [128, 128], bf16),
        ("gbd1", [128, 1], f32), ("gbd2", [128, 1], f32), ("wod", [128, 1], bf16),
        ("dsqd", [128, NBO * 128], bf16),
        ("eidx1", [128, ngr * 64], i16), ("eidx2", [128, ngr * 64], i16),
        ("P", [128, npsl * 128], bf16),
    ]:
        ext[name] = nc.dram_tensor(name, shape, dt, kind="ExternalInput").ap()
    y_ext = nc.dram_tensor("y", [128, 2 * NBO], f32, kind="ExternalOutput").ap()
    table0 = nc.dram_tensor("table0", [NP, DSL], bf16).ap()
    table1 = nc.dram_tensor("table1", [NP, DSL], bf16).ap()

    rsem = nc.alloc_semaphore("rdma_rsem")
    lsem = nc.alloc_semaphore("rdma_lsem")

    # persistent SBUF (outside tile pools): consts, L1 output, rdma rx buffer
    ct = {}
    for name in ("xt", "W1ab", "W2m", "b2c", "gwb1", "gwb2", "gbd1", "gbd2",
                 "wod", "dsqd", "eidx1", "eidx2", "P"):
        ct[name] = nc.alloc_sbuf_tensor(name + "_sb", list(ext[name].shape),
                                        ext[name].dtype).ap()
    ident = nc.alloc_sbuf_tensor("ident_sb", [128, 128], bf16).ap()
    hs1x = nc.alloc_sbuf_tensor("hs1x", [128, NBO * DSL], bf16).ap()
    rx = nc.alloc_sbuf_tensor("rx", [128, 3 * NBO * DSL], bf16).ap()

    HALF0 = 12 * DSL          # L1 blocks 0..11 ready after grp 2
    HALF1 = NBO * DSL - HALF0

    def gcn_layer(tc, L, tbl_in, eidx, gwb, gbd, hs_self, hs_out, y_nb):
        with tc.tile_pool(name=f"gg{L}", bufs=8) as gp, \
             tc.tile_pool(name=f"gv{L}", bufs=4) as wv, \
             tc.tile_pool(name=f"gps{L}", bufs=4, space="PSUM") as qs, \
             tc.tile_pool(name=f"gpw{L}", bufs=2, space="PSUM") as qw, \
             tc.tile_pool(name=f"gpt{L}", bufs=2, space="PSUM") as qt:
            gtiles = {}

            ngrp = (nchp + GSZ - 1) // GSZ

            def ensure_gathered(ci_needed):
                gi = ci_needed // GSZ
                if gi not in gtiles:
                    nci = min(GSZ, nchp - gi * GSZ)
                    gt = gp.tile([128, GSZ * DSL], bf16, tag="g")
                    nc.gpsimd.dma_gather(
                        out_ap=gt[:, 0:nci * DSL].rearrange(
                            "p (c e) -> p c e", c=nci),
                        in_ap=tbl_in[:],
                        idxs_ap=eidx[:, 64 * gi:64 * gi + 8 * nci],
                        num_idxs=nci * 128,
                        num_idxs_reg=nci * 128,
                        elem_size=DSL,
                    )
                    gtiles[gi] = gt
                return gtiles[gi]

            for grp in range(NBO // 4):
                ps = qs.tile([128, 512], f32, tag="ps", space="PSUM")
                for bl in range(4):
                    i = 4 * grp + bl
                    reg = ps[:, 128 * bl:128 * (bl + 1)]
                    ovl = sched[i]
                    nc.tensor.matmul(
                        reg,
                        lhsT=hs_self[:, 128 * i:128 * (i + 1)],
                        rhs=ct["dsqd"][:, 128 * i:128 * (i + 1)],
                        start=True, stop=(len(ovl) == 0))
                    for j, (cidx, k0, k1) in enumerate(ovl):
                        gt = ensure_gathered(cidx)
                        slot = cidx % GSZ
                        pidx = psl_of[i] + j
                        nc.tensor.matmul(
                            reg,
                            lhsT=gt[:, DSL * slot:DSL * (slot + 1)],
                            rhs=ct["P"][:, 128 * pidx:128 * (pidx + 1)],
                            start=False, stop=(j == len(ovl) - 1))
                st = wv.tile([128, 512], bf16, tag="st")
                nc.scalar.activation(st[:], ps[:],
                                     mybir.ActivationFunctionType.Copy)
                pw = qw.tile([128, 512], f32, tag="pw", space="PSUM")
                nc.tensor.matmul(pw[:], lhsT=gwb[:], rhs=st[:],
                                 start=True, stop=True)
                hn = wv.tile([128, 512], bf16, tag="hn")
                nc.scalar.activation(hn[:], pw[:],
                                     mybir.ActivationFunctionType.Relu,
                                     bias=gbd[:, 0:1])
                if L == 0:
                    pt = qt.tile([128, 512], bf16, tag="pt2", space="PSUM")
                    for bl in range(4):
                        nc.tensor.transpose(
                            pt[:, 128 * bl:128 * (bl + 1)],
                            hn[:, 128 * bl:128 * (bl + 1)], ident[:])
                    ns = slice(512 * grp, 512 * (grp + 1))
                    nc.vector.tensor_copy(hs_out[:, ns], pt[:])
                    nc.sync.dma_start(
                        table1[ns, :].rearrange("(a p) d -> p a d", p=128),
                        hs_out[:, ns].rearrange("p (a d) -> p a d", a=4))
                    if grp == 2:
                        # first-half shards ready: overlap sends with L1 tail
                        for j, dd in enumerate((2, 6, 4)):
                            rdests = [None] * 8
                            rdests[dd] = (0, dd)
                            nc.gpsimd.remote_dma_broadcast(
                                rx[:, j * NBO * DSL:j * NBO * DSL + HALF0],
                                hs_out[:, 0:HALF0], rsem, lsem, rdests=rdests)
                        nc.gpsimd.trigger_dma(count=3)
                else:
                    yp = qt.tile([128, 8], f32, tag="yp", space="PSUM")
                    for bl in range(4):
                        for s in range(2):
                            nc.tensor.matmul(
                                yp[:, 2 * bl + s:2 * bl + s + 1],
                                lhsT=hn[64 * s:64 * (s + 1),
                                        128 * bl:128 * (bl + 1)],
                                rhs=ct["wod"][64 * s:64 * (s + 1), :],
                                start=True, stop=True)
                    nc.vector.tensor_scalar_add(
                        y_nb[:, 8 * grp:8 * (grp + 1)], yp[:], bo_f)
            if L == 0:
                for j, dd in enumerate((2, 6, 4)):
                    rdests = [None] * 8
                    rdests[dd] = (0, dd)
                    nc.gpsimd.remote_dma_broadcast(
                        rx[:, j * NBO * DSL + HALF0:(j + 1) * NBO * DSL],
                        hs_out[:, HALF0:], rsem, lsem, rdests=rdests)
                nc.gpsimd.trigger_dma(count=3)

    with tile.TileContext(nc) as tc:
        with tc.tile_pool(name="hs", bufs=1) as hp:
            late = ("eidx2", "gwb2", "gbd2", "wod")
            for name in ct:
                if name in late:
                    continue
                if name == "P":
                    half = (npsl // 2) * 128
                    nc.sync.dma_start(ct["P"][:, 0:half], ext["P"][:, 0:half])
                else:
                    nc.sync.dma_start(ct[name][:], ext[name][:])
            make_identity(nc, ident[:])
            hs0 = hp.tile([128, NBLK * DSL], bf16, tag="hs0")

            # ---- conv stage: all 80 blocks (own-first order), 2 slices
            with tc.tile_pool(name="cv", bufs=4) as vp, \
                 tc.tile_pool(name="cvp", bufs=2, space="PSUM") as pp, \
                 tc.tile_pool(name="cvp2", bufs=2, space="PSUM") as pp2:
                for g in range(NBLK // 4):
                    ns = slice(512 * g, 512 * (g + 1))
                    ph1 = pp.tile([128, 1024], f32, tag="ph1", space="PSUM")
                    for s in range(2):
                        for bl in range(4):
                            nc.tensor.matmul(
                                ph1[:, 512 * s + 128 * bl:512 * s + 128 * (bl + 1)],
                                lhsT=ct["W1ab"][:],
                                rhs=ct["xt"][:, s * NP + 512 * g + 128 * bl:
                                             s * NP + 512 * g + 128 * (bl + 1)],
                                start=True, stop=True)
                    h1 = vp.tile([128, 1024], bf16, tag="h1")
                    nc.vector.tensor_scalar_max(h1[:], ph1[:], 0.0)
                    h2s = []
                    for s in range(2):
                        ph2 = pp.tile([64, 512], f32, tag="ph2", space="PSUM")
                        nc.tensor.matmul(ph2[:], lhsT=ct["W2m"][:],
                                         rhs=h1[:, 512 * s:512 * (s + 1)],
                                         start=True, stop=True)
                        h2 = vp.tile([64, 512], bf16, tag="h2")
                        nc.scalar.activation(h2[:], ph2[:],
                                             mybir.ActivationFunctionType.Relu,
                                             bias=ct["b2c"][:, 0:1])
                        h2s.append(h2)
                    pt = pp2.tile([128, 512], bf16, tag="pt", space="PSUM")
                    for s in range(2):
                        for bl in range(4):
                            nc.tensor.transpose(
                                pt[:, 128 * bl + 64 * s:128 * bl + 64 * (s + 1)],
                                h2s[s][:, 128 * bl:128 * (bl + 1)],
                                ident[0:64, 0:64])
                    nc.vector.tensor_copy(hs0[:, ns], pt[:])
                    nc.sync.dma_start(
                        table0[ns, :].rearrange("(a p) d -> p a d", p=128),
                        hs0[:, ns].rearrange("p (a d) -> p a d", a=4))

            half = (npsl // 2) * 128
            nc.sync.dma_start(ct["P"][:, half:], ext["P"][:, half:])
            for name in ("eidx2", "gwb2", "gbd2", "wod"):
                nc.sync.dma_start(ct[name][:], ext[name][:])

            # ---- GCN layer 1 (own quarter) + rdma shard exchange
            gcn_layer(tc, 0, table0, ct["eidx1"], ct["gwb1"], ct["gbd1"],
                      hs0, hs1x, None)

    # cross-core barrier: proves all peers fired their triggers; the 15us
    # collective latency dwarfs the ~4us in-flight shard transfers.
    bsem = nc.alloc_semaphore("bar_sem")
    bar_in = nc.dram_tensor("bar_in", [1, 1], mybir.dt.uint8).ap()
    bar_out = nc.dram_tensor("bar_out", [4, 1], mybir.dt.uint8).ap()
    nc.gpsimd.collective_compute(
        "AllGather", mybir.AluOpType.bypass,
        replica_groups=[[0, 2, 4, 6], [1, 3, 5, 7]],
        ins=[bar_in], outs=[bar_out]).then_inc(bsem, 1)
    nc.sync.wait_ge(bsem, 1)
    nc.gpsimd.wait_ge(bsem, 1)

    with tile.TileContext(nc) as tc2:
        with tc2.tile_pool(name="fin", bufs=1) as fp:
            for j in range(3):
                ns = slice((j + 1) * NBO * 128, (j + 2) * NBO * 128)
                nc.sync.dma_start(
                    table1[ns, :].rearrange("(a p) d -> p a d", p=128),
                    rx[:, j * NBO * DSL:(j + 1) * NBO * DSL].rearrange(
                        "p (a d) -> p a d", a=NBO))
            y_nb = fp.tile([128, 2 * NBO], f32, tag="ynb")
            gcn_layer(tc2, 1, table1, ct["eidx2"], ct["gwb2"], ct["gbd2"],
                      hs1x, None, y_nb)
            nc.sync.dma_start(y_ext[:], y_nb[:])
    nc.compile()
    return nc


def _run(inputs):
    from concourse.bass_utils import run_bass_kernel_spmd

    in_maps, chunks, ngr, bo_f = _host_prep(inputs)
    key = (len(chunks), ngr)
    if key not in _cache:
        _cache[key] = _build(chunks, ngr, bo_f)
    nc = _cache[key]

    res = run_bass_kernel_spmd(nc, in_maps, list(range(8)))
    y = np.zeros((B, N), dtype=np.float32)
    for c in range(8):
        sg, q = c % 2, c // 2
        y_nb = res.results[c]["y"]       # [128, 2*NBO], col 2i+s
        for i in range(NBO):
            lo = q * 2560 + i * 128
            hi = min(lo + 128, N)
            if hi <= lo:
                continue
            for s in range(2):
                y[2 * sg + s, lo:hi] = y_nb[:hi - lo, 2 * i + s]
    return y


def kernel(**inputs):
    return _run(inputs)
